# revision 27
# baseline (speedup 1.0000x reference)
"""Causal self-attention on 8 TRN2 NeuronCores (bf16 rewrite).

Problem: B=4, T=2048, C=1024, NH=16, HD=64.
  qkv = x @ w_qkv ; per-head causal softmax attention ; y @ w_proj

Key differences vs the fp32r baseline (983 us):
  - x is transposed + cast to bf16 on the HOST (pure input-layout prep, like
    the baseline's mask/weight prep), eliminating 512 PE transposes per core
    that ran cold (HAM K=4/8) and serialized the whole front end.
  - every matmul is bf16 (1 cycle/row, FWL weight loads); tolerance is 2e-2
    and bf16 end-to-end lands ~2e-3.
  - both heads' scores live in one [128, 2, 512] PSUM tile so one ACT exp
    instruction covers both heads; exp/mask/y-matmuls are column-restricted
    to the causal range (no wasted exp work below the diagonal blocks).
  - softmax denominator: ones-column in the v matrix rides the y-matmul;
    1/s via nc.vector.reciprocal (no Ln/Exp activation-table thrash), then
    a K=1 PE matmul broadcasts it across the 64 head dims.
  - qkv for batch b+1 is emitted interleaved with attention of batch b so
    the PE stays dense (warm) while ACT paces the softmax.
  - AllToAll exchanges bf16 (half the collective bytes), two halves so the
    first one overlaps the attention tail.
"""

import numpy as np
import ml_dtypes

import concourse.bass as bass
import concourse.mybir as mybir
import concourse.tile as tile
from concourse import bacc
from concourse.bass_utils import run_bass_kernel_spmd
from concourse.masks import make_identity

B, T, C = 4, 2048, 1024
NH, HD = 16, 64
NCORES = 8
HPC = NH // NCORES          # heads per core = 2
D2 = HPC * HD               # 128 head-dims per core
ROWS = B * T                # 8192 flattened rows
RSL = ROWS // NCORES        # 1024 rows per core slice
P = 128
QTL = 512                   # q tile
NJ = T // QTL               # 4 q-tiles per batch
KVC = T // P                # 16 kv chunks of 128 per batch
NRC = 4                     # row chunks (of 512) per batch for qkv
CK = C // P                 # 8 contraction chunks
SCALE = 1.0 / np.sqrt(HD)

F32 = mybir.dt.float32
F32R = mybir.dt.float32r
BF16 = mybir.dt.bfloat16
AF = mybir.ActivationFunctionType
ALU = mybir.AluOpType

_CACHED_NC = None
LAST_RESULTS = None  # BassKernelResults of the most recent launch (for profiling)

try:  # reuse compiled executables across calls/processes when supported
    import jax

    jax.config.update("jax_compilation_cache_dir", "/tmp/jax_cache")
    jax.config.update("jax_persistent_cache_min_compile_time_secs", 1.0)
except Exception:
    pass


def _build_nc():
    nc = bacc.Bacc(None, target_bir_lowering=False, num_devices=NCORES)

    xT_in = nc.dram_tensor("xT", [C, ROWS], BF16, kind="ExternalInput")
    wq_in = nc.dram_tensor("wq", [P, CK, D2], BF16, kind="ExternalInput")
    wk_in = nc.dram_tensor("wk", [P, CK, D2], BF16, kind="ExternalInput")
    wv_in = nc.dram_tensor("wv", [P, CK, D2], BF16, kind="ExternalInput")
    wp_in = nc.dram_tensor("wp", [P, CK, C], BF16, kind="ExternalInput")
    tri_in = nc.dram_tensor("tri", [P, HPC, P], BF16, kind="ExternalInput")
    e2_in = nc.dram_tensor("e2", [NH, CK, P], BF16, kind="ExternalInput")
    outT = nc.dram_tensor("outT", [C, RSL], F32, kind="ExternalOutput")

    rg = [list(range(NCORES))]

    with tile.TileContext(nc) as tc:
        with (
            tc.tile_pool(name="persist", bufs=1) as pp,
            tc.tile_pool(name="dram", bufs=1, space="DRAM") as dram,
            tc.tile_pool(name="work", bufs=1) as pw,
            tc.tile_pool(name="psum", bufs=1, space="PSUM") as ps,
        ):
            # ---- DRAM collective buffers (two halves: rows 0:512 / 512:1024
            # of every core's slice) ----
            # y rows 0:128, softmax denominators in rows 128:130
            a2a_in0 = dram.tile([NCORES, D2 + HPC, RSL // 2], BF16)
            a2a_in1 = dram.tile([NCORES, D2 + HPC, RSL // 2], BF16)
            a2a_out0 = dram.tile([NCORES, D2 + HPC, RSL // 2], BF16)
            a2a_out1 = dram.tile([NCORES, D2 + HPC, RSL // 2], BF16)

            ident = pp.tile([P, P], BF16)
            make_identity(nc, ident[:])

            tri_sb = pp.tile([P, HPC, P], BF16)
            nc.sync.dma_start(tri_sb[:], tri_in[:])

            # persistent SBUF activations (bf16, feature-major q/k)
            qT = pp.tile([P, ROWS], BF16)
            kT = pp.tile([P, ROWS], BF16)
            # v row-major chunks + ones column for the softmax denominator:
            # vaug[:, ch, h, 0:64] = v rows, vaug[:, ch, h, 64] = 1.0
            vaug = pp.tile([P, NRC * B * 4, HPC, HD + 1], BF16)
            ones_f = pp.tile([P, NRC * B * 4], F32)
            nc.vector.memset(ones_f[:], 1.0)
            nc.vector.tensor_copy(vaug[:, :, 0, HD], ones_f[:])
            nc.vector.tensor_copy(vaug[:, :, 1, HD], ones_f[:])

            # head-selector matrix for the post-a2a 1/s broadcast:
            # e2[i, kk, d] = 1 iff i == 2*kk + (d // 64)
            e2_sb = pp.tile([NH, CK, P], BF16)
            nc.sync.dma_start(e2_sb[:], e2_in[:])

            # qkv weights -> bf16 [P, CK, D2] (host-packed, one DMA each)
            w_sb = {}
            for nm, wt in (("q", wq_in), ("k", wk_in), ("v", wv_in)):
                wsb = pp.tile([P, CK, D2], BF16, name=f"w_{nm}")
                nc.sync.dma_start(wsb[:], wt[:])
                w_sb[nm] = wsb
            wp_sb = pp.tile([P, CK, C], BF16)  # loaded late, see below

            # ---------------- emission helpers ----------------

            def emit_qkv_rc(b, rc):
                """qkv + v-transpose for 512 rows (row-chunk rc of batch b)."""
                n = b * NRC + rc
                n0 = n * QTL
                xts = []
                for co in range(CK):
                    xt = pw.tile([P, QTL], BF16, tag=f"xt{co}", bufs=2,
                                 name=f"xt{co}")
                    nc.sync.dma_start(
                        xt[:], xT_in[co * P : (co + 1) * P, n0 : n0 + QTL]
                    )
                    xts.append(xt)
                for nm, dstT in (("q", qT), ("k", kT), ("v", None)):
                    acc = ps.tile([P, QTL], F32, tag="qkv", bufs=2, name="acc")
                    for ko in range(CK):
                        nc.tensor.matmul(
                            acc[:], w_sb[nm][:, ko, :], xts[ko][:],
                            start=(ko == 0), stop=(ko == CK - 1),
                        )
                    if dstT is not None:
                        nc.vector.tensor_copy(dstT[:, n0 : n0 + QTL], acc[:])
                    else:
                        vtmp = pw.tile([P, QTL], BF16, tag="vtmp", bufs=2)
                        nc.vector.tensor_copy(vtmp[:], acc[:])
                        tv = ps.tile([P, 4, P], BF16, tag="qkv", bufs=2, name="tv")
                        for s in range(4):
                            nc.tensor.transpose(
                                tv[:, s, :], vtmp[:, s * P : (s + 1) * P], ident[:]
                            )
                        ch0 = 4 * n
                        for h in range(HPC):
                            nc.vector.tensor_copy(
                                vaug[:, ch0 : ch0 + 4, h, 0:HD],
                                tv[:, :, h * HD : (h + 1) * HD],
                            )

            def emit_attn_chunks(b, j):
                """score+exp+mask+y matmuls for q-tile j of batch b."""
                q0 = (b * NJ + j) * QTL
                nkv = 4 * j + 4
                ps_ys = [
                    ps.tile([HD + 1, QTL], F32, tag="y", bufs=2,
                            name=f"ps_y{h}")
                    for h in range(HPC)
                ]
                for i in range(nkv):
                    ch = b * KVC + i
                    m = i - 4 * j
                    c0 = max(m, 0) * P          # first causal q column
                    ncol = QTL - c0
                    ps_s = ps.tile([P, HPC, QTL], F32, tag="s", bufs=2,
                                   name="ps_s")
                    for h in range(HPC):
                        hsl = slice(h * HD, (h + 1) * HD)
                        nc.tensor.matmul(
                            ps_s[:, h, c0:QTL],
                            kT[hsl, ch * P : (ch + 1) * P],
                            qT[hsl, q0 + c0 : q0 + QTL],
                            start=True, stop=True,
                        )
                    att = pw.tile([P, HPC, QTL], BF16, tag="att", bufs=3)
                    nc.scalar.activation(
                        att[:, :, c0:QTL], ps_s[:, :, c0:QTL], AF.Exp,
                        scale=float(SCALE),
                    )
                    if m >= 0:
                        nc.vector.tensor_tensor(
                            att[:, :, c0 : c0 + P],
                            att[:, :, c0 : c0 + P],
                            tri_sb[:],
                            ALU.mult,
                        )
                    for h in range(HPC):
                        nc.tensor.matmul(
                            ps_ys[h][:, c0:QTL],
                            vaug[:, ch, h, :],
                            att[:, h, c0:QTL],
                            start=(i == 0), stop=(i == nkv - 1),
                        )
                return ps_ys

            def emit_stage_y(b, j, ps_ys):
                """copy unnormalized y + s rows to SBUF and DMA them into the
                AllToAll staging buffers; normalization happens post-a2a where
                the s rows land partition-packed (parallel reciprocal)."""
                row0 = b * T + j * QTL
                s = row0 // RSL
                ybuf = a2a_in0 if (row0 % RSL) == 0 else a2a_in1
                for h in range(HPC):
                    yraw = pw.tile([HD + 1, QTL], BF16, tag=f"yraw{h}", bufs=2,
                                   name=f"yraw{h}")
                    nc.vector.tensor_copy(yraw[:], ps_ys[h][0 : HD + 1, :])
                    nc.sync.dma_start(
                        ybuf[s, h * HD : (h + 1) * HD, :], yraw[0:HD, :]
                    )
                    nc.sync.dma_start(
                        ybuf[s, D2 + h, :], yraw[HD : HD + 1, :]
                    )

            def emit_proj_half(s2):
                a2a_out = a2a_out0 if s2 == 0 else a2a_out1
                # 1/s for all 16 heads at once: partition-packed reciprocal
                s_sb = pw.tile([NH, QTL], BF16, tag="s_sb", bufs=2)
                for kk in range(NCORES):
                    nc.sync.dma_start(
                        s_sb[HPC * kk : HPC * (kk + 1), :],
                        a2a_out[kk, D2 : D2 + HPC, :],
                    )
                rec_sb = pw.tile([NH, QTL], BF16, tag="rec_sb", bufs=2)
                with nc.allow_low_precision("1/s at bf16, tol 2e-2"):
                    nc.vector.reciprocal(rec_sb[:], s_sb[:])
                yns = []
                for kk in range(NCORES):
                    yr = pw.tile([P, QTL], BF16, tag=f"yr{kk}", bufs=2,
                                 name=f"yr{kk}")
                    nc.sync.dma_start(yr[:], a2a_out[kk, 0:D2, :])
                    ps_bc = ps.tile([P, QTL], F32, tag="s", bufs=2,
                                    name="ps_bc")
                    nc.tensor.matmul(
                        ps_bc[:], e2_sb[:, kk, :], rec_sb[:],
                        start=True, stop=True,
                    )
                    yn = pw.tile([P, QTL], BF16, tag=f"yn{kk}", bufs=2,
                                 name=f"yn{kk}")
                    nc.vector.tensor_tensor(yn[:], yr[:], ps_bc[:], ALU.mult)
                    yns.append(yn)
                for oc in range(CK):
                    ps_o = ps.tile([P, QTL], F32, tag="qkv", bufs=2,
                                   name="ps_o")
                    for kk in range(NCORES):
                        nc.tensor.matmul(
                            ps_o[:],
                            wp_sb[:, kk, oc * P : (oc + 1) * P],
                            yns[kk][:],
                            start=(kk == 0), stop=(kk == NCORES - 1),
                        )
                    osb = pw.tile([P, QTL], F32, tag="osb", bufs=2)
                    nc.vector.tensor_copy(osb[:], ps_o[:])
                    nc.sync.dma_start(
                        outT[oc * P : (oc + 1) * P, s2 * QTL : (s2 + 1) * QTL],
                        osb[:],
                    )

            # ---------------- main emission ----------------
            # C(b, j) only needs q/k/v rows <= (j+1)*512 of batch b, so each
            # B row-chunk rc_j is deferred to just before C(b, j): every j
            # boundary gets ~5us of dense PE filler, and the previous j's
            # normalize (reciprocal chain) hides behind it.
            def emit_a2a(half):
                ins_y = a2a_in0 if half == 0 else a2a_in1
                outs_y = a2a_out0 if half == 0 else a2a_out1
                nc.gpsimd.collective_compute(
                    "AllToAll", ALU.bypass, replica_groups=rg,
                    ins=[ins_y[:].opt()], outs=[outs_y[:].opt()],
                )

            # global q-tile groups: all batches' j=0, then j=2, then
            # j=1, then j=3. The half-0 collective (rows 0:512 of every
            # slice = j in {0,2}) fires at ~40% of attention and hides
            # completely under the j in {1,3} work.
            done_rc = [0] * B
            for gi, j in enumerate((0, 2, 1, 3)):
                for b in range(B):
                    if b == B - 1 and gi == 2:
                        nc.sync.dma_start(wp_sb[:], wp_in[:])
                    while done_rc[b] <= j:
                        emit_qkv_rc(b, done_rc[b])
                        done_rc[b] += 1
                    ps_ys = emit_attn_chunks(b, j)
                    emit_stage_y(b, j, ps_ys)
                if gi == 1:
                    emit_a2a(0)
            emit_a2a(1)
            emit_proj_half(0)
            emit_proj_half(1)

    nc.finalize()
    return nc


def _get_nc():
    global _CACHED_NC
    if _CACHED_NC is None:
        _CACHED_NC = _build_nc()
    return _CACHED_NC


def kernel(x, mask, w_qkv, w_proj):
    bf = ml_dtypes.bfloat16
    x = np.asarray(x, dtype=np.float32)
    w_qkv = np.asarray(w_qkv, dtype=np.float32)
    w_proj = np.asarray(w_proj, dtype=np.float32)

    # host-side input layout prep: feature-major bf16 activations and
    # partition-packed weights (w[ko*128+p, d] -> packed[p, ko, d])
    xT = np.ascontiguousarray(x.reshape(ROWS, C).T.astype(bf))

    def pack(w):  # [C, D] -> [P, CK, D], partition-major
        return np.ascontiguousarray(
            w.reshape(CK, P, -1).transpose(1, 0, 2).astype(bf)
        )

    wp_bf = pack(w_proj)

    # diagonal-block causal pattern (multiplicative, transposed):
    # tri[p, h, c] = keep(kv_local=p, q_local=c), identical for both heads
    mt = np.asarray(mask).reshape(T, T)[:P, :P].T.astype(bf)
    tri = np.ascontiguousarray(np.broadcast_to(mt[:, None, :], (P, HPC, P)))

    # head-selector for the post-a2a 1/s broadcast
    e2 = np.zeros((NH, CK, P), dtype=bf)
    for kk in range(CK):
        e2[2 * kk, kk, 0:HD] = 1.0
        e2[2 * kk + 1, kk, HD:P] = 1.0

    in_maps = []
    for r in range(NCORES):
        sl = slice(r * D2, (r + 1) * D2)
        in_maps.append(
            {
                "xT": xT,
                "wq": pack(w_qkv[:, sl]),
                "wk": pack(w_qkv[:, C:][:, sl]),
                "wv": pack(w_qkv[:, 2 * C:][:, sl]),
                "wp": wp_bf,
                "tri": tri,
                "e2": e2,
            }
        )

    nc = _get_nc()
    res = run_bass_kernel_spmd(nc, in_maps, core_ids=list(range(NCORES)))
    global LAST_RESULTS
    LAST_RESULTS = res

    out = np.empty((ROWS, C), dtype=np.float32)
    for r in range(NCORES):
        out[r * RSL : (r + 1) * RSL, :] = res.results[r]["outT"].T
    return out.reshape(B, T, C)
